# revision 4
# baseline (speedup 1.0000x reference)
"""Trainium2 Bass kernel for nn_MoEFusion — fp8 DoubleRow variant.

Data-parallel across 8 NeuronCores: batch (32768) sharded into 8 slices
of 4096, weights replicated. No collectives.

vs the bf16 baseline (151.5us -> ~113us):
- All heavy GEMMs (proj K=768, W1 K=384+bias, W2 K=2x128, gate K=384)
  run in fp8e4 with DoubleRow perf mode: each DR matmul consumes TWO
  K-chunks per ~220ns instruction -> PE stream work nearly halves
  (57 -> 35 matmuls per 512-column stripe).
- Features shipped fp8 (quarters HBM traffic vs f32).
- proj_b folded into exp_b1/gate_b on host (x evacs become pure Copy);
  head_b folded into the head matmul via a constant-one row in pen;
  exp_b1 folded into the W1 matmul via an fp8 bias-row chunk whose
  moving counterpart is a constant-1.0 row in xq (pad chunk must be
  zeroed: fp8 garbage decodes as NaN and 0*NaN poisons the psum).
- h evacs are then pure relus, processed TWO experts per op across
  adjacent psum banks ([128,2,512]), split ACT (pairs 0-1) / DVE (2-3).
- gw broadcast to 128 partitions via a DRAM bounce (SBUF sources
  cannot replicate across partitions at speed; a DRAM source with a
  0-stride leading dim can), issued on the idle sync ring.
- sh = h*gw pair-muls on DVE (pairs 0-2) + GPSIMD (pair 3); sh pairs
  are the fp8 [128,2,512] moving operands of the W2 DoubleRow matmuls.
- l2 (W2 accumulation) and pre/head of older stripes are emitted inside
  stripe s (4-stripe software pipeline) so the PE never waits on the
  gate-softmax/broadcast chain.
End-to-end numeric sim and HW: rel err 3.8e-3 vs the 2e-2 gate.
"""

import sys

if "/opt/trn_rl_repo" not in sys.path:
    sys.path.insert(0, "/opt/trn_rl_repo")

from contextlib import ExitStack

import ml_dtypes
import numpy as np

# ---- problem constants (hardcoded per contract) ----
B = 32768
NCORES = 8
BL = B // NCORES  # 4096 per core
STRIPE = 512
NM = 3
NE = 8
D_IN = 768
KIN = D_IN // 128  # 6
D_P = 128
D_X = 384
KX = D_X // 128  # 3

BF16 = ml_dtypes.bfloat16
F8 = ml_dtypes.float8_e4m3

# ---- fp8 packed weights ----
# wmat8p [128, 18, 128]: proj chunks (m*6 + k): proj_w[m, k*128+p, o]
# wmat8r [128, 40, 128]: W1 as 4 chunks/expert (k=0..2 weights, k=3 the
# b1 bias row: row0 = b1_eff[e], rows 1-127 zero), then W2 chunk per
# expert (pairs adjacent).
CH_PROJ = 0
CH_W1 = CH_PROJ + NM * KIN       # 18 proj chunks in wmat8p
W1CH = KX + 1                    # 4 chunks per expert incl bias row
CH_W2R = NE * W1CH               # 32 (wmat8r-local)
NCH8R = CH_W2R + NE              # 40

# gate fp8: wgate8 [128, 3, 16]: [p, k, j<8] = gate_w[k*128+p, j]
# (chunk stride padded to 16B for the DoubleRow ldweights alignment rule)

# ---- bf16 packed weights (columns of [128, WBFCOLS]) ----
OFF_PRE = 0                      # [p, o] = pre_w[p, o]
OFF_HEAD = OFF_PRE + 64          # [p<65, o]: rows 0-63 head_w, row 64 head_b
OFF_B2B = OFF_HEAD + 2           # [p<8, o] = exp_b2[p, o]
WBFCOLS = OFF_B2B + 128          # 194

# ---- f32r packed weights (softmax sum) ----
OFF_ONES = 0
WRCOLS = OFF_ONES + NE           # 8

# ---- f32 biases (columns of [128, WBCOLS]) ----
OFF_B1 = 0                       # b1_eff = exp_b1 + W1^T pb  (pb = proj_b concat)
OFF_GATEB = OFF_B1 + NE          # gate_b_eff = gate_b + gate_w^T pb
OFF_PREB = OFF_GATEB + 1
WBCOLS = OFF_PREB + 1            # 10


def pack_weights(inp):
    w8p = np.zeros((128, CH_W1, 128), np.float32)
    pw = np.asarray(inp["proj_w"], np.float32)
    w8p[:, :, :] = (
        pw.reshape(NM, KIN, 128, 128).transpose(2, 0, 1, 3).reshape(128, NM * KIN, 128)
    )
    w8p = w8p.astype(F8)
    # fold proj_b into b1 (x ships bias-free); b1 rides the 4th W1 chunk
    pb = np.asarray(inp["proj_b"], np.float32).reshape(-1)       # [384]
    w1 = np.asarray(inp["exp_w1"], np.float32)                   # [8,384,128]
    b1_eff = np.asarray(inp["exp_b1"], np.float32) + w1.transpose(0, 2, 1) @ pb
    w8r = np.zeros((128, NCH8R, 128), np.float32)
    w1c = w1.reshape(NE, KX, 128, 128).transpose(2, 0, 1, 3)     # [128,8,3,128]
    for e in range(NE):
        w8r[:, e * W1CH : e * W1CH + KX, :] = w1c[:, e, :, :]
        w8r[0, e * W1CH + KX, :] = b1_eff[e]
    w2 = np.asarray(inp["exp_w2"], np.float32)
    w8r[:, CH_W2R:, :] = w2.transpose(1, 0, 2)
    w8r = w8r.astype(F8)

    g8 = np.zeros((128, KX, 16), np.float32)
    g8[:, :, :NE] = np.asarray(inp["gate_w"], np.float32).reshape(KX, 128, NE).transpose(1, 0, 2)
    g8 = g8.astype(F8)  # [128, 3, 16], cols 8..16 zero

    wb16 = np.zeros((128, WBFCOLS), np.float32)
    wb16[:, OFF_PRE:OFF_HEAD] = np.asarray(inp["pre_w"], np.float32)
    wb16[:64, OFF_HEAD:OFF_B2B] = np.asarray(inp["head_w"], np.float32)
    wb16[64, OFF_HEAD:OFF_B2B] = np.asarray(inp["head_b"], np.float32)
    wb16[:8, OFF_B2B:WBFCOLS] = np.asarray(inp["exp_b2"], np.float32)
    wb16 = wb16.astype(BF16)

    wr = np.zeros((128, WRCOLS), np.float32)
    wr[:8, OFF_ONES:OFF_ONES + NE] = 1.0

    gb_eff = np.asarray(inp["gate_b"], np.float32) + pb @ np.asarray(
        inp["gate_w"], np.float32
    )

    wbias = np.zeros((128, WBCOLS), np.float32)
    wbias[:8, OFF_GATEB] = gb_eff
    wbias[:64, OFF_PREB] = np.asarray(inp["pre_b"], np.float32)
    return w8p, w8r, g8, wb16, wr, wbias


def build_program(n_stripes=BL // STRIPE):
    """Build the per-core Bass program (identical on all cores)."""
    import concourse.bacc as bacc
    import concourse.mybir as mybir
    import concourse.tile as tile

    f32 = mybir.dt.float32
    f32r = mybir.dt.float32r
    bf16 = mybir.dt.bfloat16
    f8 = mybir.dt.float8e4
    AF = mybir.ActivationFunctionType
    ALU = mybir.AluOpType
    DR = mybir.MatmulPerfMode.DoubleRow
    bl = n_stripes * STRIPE

    nc = bacc.Bacc(
        "TRN2",
        target_bir_lowering=False,
        debug=False,
        enable_asserts=False,
    )

    featT = nc.dram_tensor(
        "featT", [NM, n_stripes, 128, KIN, STRIPE], f8, kind="ExternalInput"
    ).ap()
    wmat8p = nc.dram_tensor("wmat8p", [128, CH_W1, 128], f8, kind="ExternalInput").ap()
    wmat8r = nc.dram_tensor(
        "wmat8r", [128, NCH8R, 128], f8, kind="ExternalInput"
    ).ap()
    wgate8 = nc.dram_tensor("wgate8", [128, KX, 16], f8, kind="ExternalInput").ap()
    wmatb = nc.dram_tensor("wmatb", [128, WBFCOLS], bf16, kind="ExternalInput").ap()
    wmatr = nc.dram_tensor("wmatr", [128, WRCOLS], f32r, kind="ExternalInput").ap()
    wbias = nc.dram_tensor("wbias", [128, WBCOLS], f32, kind="ExternalInput").ap()
    outT = nc.dram_tensor("outT", [2, bl], f32, kind="ExternalOutput").ap()
    # DRAM bounce buffer for the gate-weight partition broadcast: SBUF
    # sources cannot replicate across partitions at speed (single-partition
    # read), but a DRAM source with a 0-stride leading dim can.
    gws = nc.dram_tensor("gws", [n_stripes, NE, STRIPE], bf16, kind="Internal").ap()

    with tile.TileContext(nc) as tc, ExitStack() as ctx:
        wp_pool = ctx.enter_context(tc.tile_pool(name="wp", bufs=1))
        feat_pool = ctx.enter_context(tc.tile_pool(name="feat", bufs=9))
        x_pool = ctx.enter_context(tc.tile_pool(name="x", bufs=4))
        gw_pool = ctx.enter_context(tc.tile_pool(name="gw", bufs=6))
        grow_pool = ctx.enter_context(tc.tile_pool(name="grow", bufs=4))
        gb_pool = ctx.enter_context(tc.tile_pool(name="gb", bufs=4))
        h_pool = ctx.enter_context(tc.tile_pool(name="h", bufs=10))
        sh_pool = ctx.enter_context(tc.tile_pool(name="sh", bufs=24))
        f_pool = ctx.enter_context(tc.tile_pool(name="f", bufs=2))
        pen_pool = ctx.enter_context(tc.tile_pool(name="pen", bufs=4))
        o_pool = ctx.enter_context(tc.tile_pool(name="o", bufs=4))

        px_pool = ctx.enter_context(tc.tile_pool(name="px", bufs=2, space="PSUM"))
        ph_pool = ctx.enter_context(tc.tile_pool(name="ph", bufs=2, space="PSUM"))
        pf_pool = ctx.enter_context(tc.tile_pool(name="pf", bufs=1, space="PSUM"))
        ps_pool = ctx.enter_context(tc.tile_pool(name="ps", bufs=1, space="PSUM"))

        # preload packed weights once. proj weights lead the scalar ring so
        # the first matmuls can start ASAP; everything else follows in
        # first-use order (W1, gate, W2, pre/head/b2).
        W8p = wp_pool.tile([128, CH_W1, 128], f8)
        nc.scalar.dma_start(W8p[:], wmat8p[:])
        W8r = wp_pool.tile([128, NCH8R, 128], f8)
        nc.scalar.dma_start(W8r[:], wmat8r[:])
        G8 = wp_pool.tile([128, KX, 16], f8)
        nc.scalar.dma_start(G8[:], wgate8[:])
        Wbt = wp_pool.tile([128, WBFCOLS], bf16)
        nc.scalar.dma_start(Wbt[:], wmatb[:])
        Bz = wp_pool.tile([128, WBCOLS], f32)
        nc.scalar.dma_start(Bz[:], wbias[:])
        Wr = wp_pool.tile([128, WRCOLS], f32r)
        nc.scalar.dma_start(Wr[:], wmatr[:])

        def w8r(c, n):
            return W8r[:, c : c + n, :]

        def wb(off, n, parts=128):
            return Wbt[:parts, off : off + n]

        def bslice(off, parts=128):
            return Bz[:parts, off : off + 1]

        pends = []  # (sh_pairs, gwT, bsl) of the previous three stripes
        head_pend = None

        def emit_l2(pend):
            sh, gwT, bsl = pend
            pf = pf_pool.tile([128, STRIPE], f32, tag="pf")
            nc.tensor.matmul(
                pf[:], wb(OFF_B2B, 128, parts=8), gwT[:],
                start=True, stop=False,
            )
            for pi in range(NE // 2):
                nc.tensor.matmul(
                    pf[:],
                    w8r(CH_W2R + 2 * pi, 2),
                    sh[pi][:],
                    start=False,
                    stop=(pi == NE // 2 - 1),
                    perf_mode=DR,
                )
            fT = f_pool.tile([128, STRIPE], bf16, tag="f")
            nc.scalar.copy(fT[:], pf[:])
            return fT

        def emit_pre(fT):
            pp = ps_pool.tile([64, STRIPE], f32, tag="ps")
            nc.tensor.matmul(pp[:], wb(OFF_PRE, 64), fT[:],
                             start=True, stop=True)
            pen = pen_pool.tile([65, STRIPE], bf16, tag="pen")
            nc.scalar.activation(
                pen[:64, :], pp[:], AF.Relu, bias=bslice(OFF_PREB, parts=64),
                scale=1.0,
            )
            nc.gpsimd.memset(pen[64:65, :], 1.0)
            return pen

        def emit_head2(pen, bsl):
            po = ps_pool.tile([2, STRIPE], f32, tag="ps")
            nc.tensor.matmul(po[:], wb(OFF_HEAD, 2, parts=65), pen[:],
                             start=True, stop=True)
            ot = o_pool.tile([2, STRIPE], f32, tag="o")
            nc.scalar.activation(ot[:], po[:], AF.Copy)
            nc.sync.dma_start(outT[:, bsl], ot[:])

        for s in range(n_stripes):
            bsl = slice(s * STRIPE, (s + 1) * STRIPE)

            # ---- load features (0.375 MB per modality, fp8, contiguous) ----
            ft = []
            for m in range(NM):
                ta = feat_pool.tile([128, KIN, STRIPE], f8, tag="feat")
                nc.sync.dma_start(ta[:], featT[m, s])
                ft.append(ta)

            # ---- per-modality projection -> xq chunks (3 DR each);
            # chunk 3 row 0 carries the constant 1.0 for the b1 bias row ----
            xq = x_pool.tile([128, KX + 1, STRIPE], f8, tag="x")
            nc.gpsimd.memset(xq[:, KX, :], 0.0)
            nc.gpsimd.memset(xq[0:1, KX, :], 1.0)
            for m in range(NM):
                px = px_pool.tile([128, STRIPE], f32, tag="px")
                for j in range(KIN // 2):
                    nc.tensor.matmul(
                        px[:],
                        W8p[:, m * KIN + 2 * j : m * KIN + 2 * j + 2, :],
                        ft[m][:, 2 * j : 2 * j + 2, :],
                        start=(j == 0), stop=(j == KIN // 2 - 1), perf_mode=DR,
                    )
                nc.scalar.activation(xq[:, m, :], px[:], AF.Copy)

            # ---- gate: softmax over 8 experts ----
            pg = ps_pool.tile([8, STRIPE], f32, tag="ps")
            nc.tensor.matmul(pg[:], G8[:, 0:2, :NE], xq[:, 0:2, :],
                             start=True, stop=False, perf_mode=DR)
            nc.tensor.matmul(pg[:], G8[:, 2, :NE], xq[:, 2, :],
                             start=False, stop=True)
            eT = gw_pool.tile([8, STRIPE], f32r, tag="eT")
            nc.scalar.activation(
                eT[:], pg[:], AF.Exp, bias=bslice(OFF_GATEB, parts=8), scale=1.0
            )
            psum_s = ps_pool.tile([8, STRIPE], f32, tag="ps")
            nc.tensor.matmul(psum_s[:], Wr[:8, OFF_ONES:OFF_ONES + NE], eT[:],
                             start=True, stop=True)
            rT = gw_pool.tile([8, STRIPE], f32, tag="rT")
            nc.vector.reciprocal_approx_fast(rT[:], psum_s[:])
            gwT = gw_pool.tile([8, STRIPE], bf16, tag="gwT")
            nc.vector.tensor_mul(gwT[:], eT[:], rT[:])

            # broadcast gw to all 128 partitions via a DRAM bounce: write
            # gwT rows out (parallel-partition write), read back with a
            # 0-stride leading dim (full-bandwidth replicated read).
            nc.sync.dma_start(gws[s], gwT[:])
            gball = gb_pool.tile([128, NE, STRIPE], bf16, tag="gb")
            nc.sync.dma_start(
                gball[:], gws[s].unsqueeze(0).broadcast_to([128, NE, STRIPE])
            )

            # ---- finish head of an older stripe ----
            if head_pend is not None:
                emit_head2(*head_pend)
                head_pend = None

            # ---- stage-2, four stripes back: l2 accumulation ----
            fT_prev = None
            if len(pends) == 4:
                p0 = pends.pop(0)
                fT_prev = emit_l2(p0)
                pend_bsl = p0[2]

            # ---- experts: b1 folded into the matmul; paired relu evacs
            # (ACT pairs 0-1, DVE pairs 2-3); sh pair muls DVE 0-2, pool 3 ----
            sh = []
            for pi in range(NE // 2):
                php = ph_pool.tile([128, 2, STRIPE], f32, tag="ph")
                for sub in range(2):
                    e = 2 * pi + sub
                    c0 = e * W1CH
                    nc.tensor.matmul(php[:, sub, :], w8r(c0, 2), xq[:, 0:2, :],
                                     start=True, stop=False, perf_mode=DR)
                    nc.tensor.matmul(php[:, sub, :], w8r(c0 + 2, 2),
                                     xq[:, 2:4, :],
                                     start=False, stop=True, perf_mode=DR)
                hp = h_pool.tile([128, 2, STRIPE], bf16, tag="h")
                if pi < 2:
                    nc.scalar.activation(hp[:], php[:], AF.Relu)
                else:
                    nc.vector.tensor_scalar(hp[:], php[:], 0.0, None, ALU.max)
                shp = sh_pool.tile([128, 2, STRIPE], f8, tag="sh")
                sh.append(shp)
                if pi < 3:
                    nc.vector.tensor_mul(shp[:], hp[:],
                                         gball[:, 2 * pi : 2 * pi + 2, :])
                else:
                    nc.gpsimd.tensor_mul(shp[:], hp[:], gball[:, 6:8, :])

            if fT_prev is not None:
                head_pend = (emit_pre(fT_prev), pend_bsl)
            pends.append((sh, gwT, bsl))

        if head_pend is not None:
            emit_head2(*head_pend)
        prev = None
        for p0 in pends:
            fT = emit_l2(p0)
            if prev is not None:
                emit_head2(*prev)
            prev = (emit_pre(fT), p0[2])
        emit_head2(*prev)

    nc.compile()
    return nc


_PROGRAM = None


def _get_program():
    global _PROGRAM
    if _PROGRAM is None:
        _PROGRAM = build_program()
    return _PROGRAM


def make_in_maps(inputs):
    """Host-side shard + layout prep: list of 8 per-core input maps."""
    w8p, w8r, g8, wb16, wr, wbias = pack_weights(inputs)
    feats = [
        np.asarray(inputs["feat_text"], np.float32),
        np.asarray(inputs["feat_audio"], np.float32),
        np.asarray(inputs["feat_video"], np.float32),
    ]
    n_stripes = BL // STRIPE
    in_maps = []
    for c in range(NCORES):
        sl = slice(c * BL, (c + 1) * BL)
        # [s, p, k, b]: element = f[sl][s*512 + b, k*128 + p]
        featT = np.stack([
            f[sl].reshape(n_stripes, STRIPE, KIN, 128).transpose(0, 3, 2, 1)
            for f in feats
        ])
        in_maps.append({
            "featT": np.ascontiguousarray(featT).astype(F8),
            "wmat8p": w8p,
            "wmat8r": w8r,
            "wgate8": g8,
            "wmatb": wb16,
            "wmatr": wr,
            "wbias": wbias,
        })
    return in_maps


def run_on_hw(inputs, trace=False):
    from concourse.bass_utils import run_bass_kernel_spmd

    nc = _get_program()
    in_maps = make_in_maps(inputs)
    res = run_bass_kernel_spmd(
        nc, in_maps, core_ids=list(range(NCORES)), trace=trace
    )
    out = np.concatenate([r["outT"].T for r in res.results], axis=0)
    return out, res


def kernel(**inputs):
    out, _ = run_on_hw(inputs, trace=False)
    return out
